# revision 8
# baseline (speedup 1.0000x reference)
"""Cosformer multi-head attention kernel for 8 Trainium2 NeuronCores.

Reference computation (per batch b):
    q = relu(x @ Wq.T); k = relu(x @ Wk.T); v = x @ Wv.T
    q_ = [q*sin, q*cos]; k_ = [k*sin, k*cos]      (sin/cos indexed by position n)
    kv = k_.T @ v;  z = 1/clip(q_ @ sum_n k_, 1e-6)
    out = (q_ @ kv) * z   followed by a head-scrambling reshape.

Sharding: 8 cores = 4 batches x 2 halves of the output columns (m). Each core
computes q and k in full for its batch (needed for the full-feature contraction)
and v / kv / out only for its 256-column half. No collectives needed.

Decomposition used on-chip (avoids materializing the 2D-wide q_/k_):
    kv_s = (k*sin).T @ v ; kv_c = (k*cos).T @ v            [512, 256] each
    ksum_s = sum_n (k*sin) ; ksum_c = sum_n (k*cos)        [512]
    o_s = q @ kv_s ; o_c = q @ kv_c ; qs_s = q @ ksum_s ; qs_c = q @ ksum_c
    out = (sin*o_s + cos*o_c) / clip(sin*qs_s + cos*qs_c, 1e-6)

All matmuls run as float32r (FP22 multiplies, fp32 accumulate) which is 4x
faster than true fp32 on the PE array at free-dim >= 256. The walrus verifier
requires every tensor consumed by an FP32r matmul to be declared float32r at
its producer, so all matmul operands are float32r end-to-end (bit-compatible
with fp32 on the host side).

The final head-scramble permutation is folded into the output DMA access
pattern; the host only does cheap reshapes to reassemble the full output.
"""

import math
import numpy as np

import concourse.bass as bass
import concourse.mybir as mybir
import concourse.tile as tile
from concourse import bacc
from concourse.bass import ts, ds
from concourse.bass_utils import run_bass_kernel_spmd

B, N, D = 4, 4096, 512
MH = 256          # per-core output column half width
NT = N // 128     # 32 n-tiles
DT = D // 128     # 4 d-tiles
F32 = mybir.dt.float32
F32R = mybir.dt.float32r
AF = mybir.ActivationFunctionType
ALU = mybir.AluOpType


def f32(ap):
    """Read an f32r tensor as plain fp32 (same bits) for non-matmul ops."""
    return ap.bitcast(F32)


def build_program():
    nc = bacc.Bacc("TRN2", target_bir_lowering=False, debug=False, num_devices=8)

    xT = nc.dram_tensor("xT", [D, N], F32R, kind="ExternalInput").ap()
    wqT = nc.dram_tensor("wqT", [D, D], F32R, kind="ExternalInput").ap()
    wkT = nc.dram_tensor("wkT", [D, D], F32R, kind="ExternalInput").ap()
    wvT = nc.dram_tensor("wvT", [D, MH], F32R, kind="ExternalInput").ap()
    sct = nc.dram_tensor("sct", [128, 2 * NT], F32R, kind="ExternalInput").ap()
    ident = nc.dram_tensor("ident", [128, 128], F32R, kind="ExternalInput").ap()
    out = nc.dram_tensor("out", [N // 2, D], F32, kind="ExternalOutput").ap()

    # scrambled output view: tile rows rr = h*512 + s, cols cc = jj*64 + hd
    # land at out[s*4 + jj, h*64 + hd]
    out_r = out.rearrange("(s j) (h hd) -> s j h hd", j=4, hd=64)

    with tile.TileContext(nc) as tc:
        with (
            tc.tile_pool(name="consts", bufs=1) as consts,
            tc.tile_pool(name="xpool", bufs=1) as xpool,
            tc.tile_pool(name="qpool", bufs=1) as qpool,
            tc.tile_pool(name="work", bufs=2) as work,
            tc.tile_pool(name="opool", bufs=3) as opool,
            tc.tile_pool(name="ppool", bufs=3, space="PSUM") as ppool,
            tc.tile_pool(name="kvpool", bufs=1, space="PSUM") as kvpool,
        ):
            # ---- constants ----
            # Small/weight loads go on the ACT HWDGE ring, x chunks on the SP
            # ring, so the first k/v matmuls (need sct+wk+wv+xc0) start ASAP.
            sct_sb = consts.tile([128, 2 * NT], F32R)
            nc.scalar.dma_start(sct_sb, sct)
            wk_sb = consts.tile([128, DT, D], F32R)
            nc.scalar.dma_start(wk_sb, wkT.rearrange("(dt p) m -> p dt m", p=128))
            wv_sb = consts.tile([128, DT, MH], F32R)
            nc.scalar.dma_start(wv_sb, wvT.rearrange("(dt p) m -> p dt m", p=128))

            # ---- x, 8 chunks of [128, 4, 512] (1 MiB DMAs) ----
            xc = []
            xT_r = xT.rearrange("(dt p) n -> p dt n", p=128)
            for c in range(8):
                t_ = xpool.tile([128, DT, 512], F32R, tag=f"xc{c}", name=f"xc{c}")
                nc.sync.dma_start(t_, xT_r[:, :, ts(c, 512)])
                xc.append(t_)

            wq_sb = consts.tile([128, DT, D], F32R)
            nc.scalar.dma_start(wq_sb, wqT.rearrange("(dt p) m -> p dt m", p=128))
            id_sb = consts.tile([128, 128], F32R)
            nc.scalar.dma_start(id_sb, ident)

            # qT storage [m-part, mt, n] chunks
            qc = [
                qpool.tile([128, DT, 512], F32R, tag=f"qc{c}", name=f"qc{c}")
                for c in range(8)
            ]

            # persistent psum accumulators
            kv_ps = [
                kvpool.tile([128, 512], F32, tag=f"kv{d2}", name=f"kv{d2}")
                for d2 in range(DT)
            ]
            ksum_ps = kvpool.tile([2, 512], F32, tag="ksum", name="ksum")

            # ---- fused loop: k/v/q projections + kv/ksum accumulation ----
            for t in range(NT):
                xt = xc[t // 4]
                nslc = ts(t % 4, 128)  # n-tile slice inside the x chunk
                sin_ap = f32(sct_sb[:, 2 * t : 2 * t + 1])
                cos_ap = f32(sct_sb[:, 2 * t + 1 : 2 * t + 2])

                k_ps = ppool.tile([128, 512], F32, tag="proj", name=f"k_ps{t}")
                for dt in range(DT):
                    nc.tensor.matmul(
                        k_ps, xt[:, dt, nslc], wk_sb[:, dt, :],
                        start=dt == 0, stop=dt == DT - 1,
                    )
                v_ps = ppool.tile([128, 512], F32, tag="proj", name=f"v_ps{t}")
                for dt in range(DT):
                    nc.tensor.matmul(
                        v_ps[:, :MH], xt[:, dt, nslc], wv_sb[:, dt, :],
                        start=dt == 0, stop=dt == DT - 1,
                    )
                # q-units lag 4 tiles so the first tiles only need xc0+wk+wv
                # (earlier PE start while the later x chunks stream in).
                if t >= 4:
                    u = t - 4
                    mt, nc2 = u % 4, u // 4
                    q_ps = ppool.tile([128, 512], F32, tag="proj", name=f"q_ps{u}")
                    for dt in range(DT):
                        nc.tensor.matmul(
                            q_ps, wq_sb[:, dt, ts(mt, 128)], xc[nc2][:, dt, :],
                            start=dt == 0, stop=dt == DT - 1,
                        )
                    nc.scalar.activation(qc[nc2][:, mt, :], q_ps, AF.Relu)

                k_s = work.tile([128, 512], F32R, tag="ks", name=f"ks{t}")
                nc.scalar.activation(k_s, k_ps, AF.Relu, scale=sin_ap)
                k_c = work.tile([128, 512], F32R, tag="kc", name=f"kc{t}")
                nc.scalar.activation(k_c, k_ps, AF.Relu, scale=cos_ap)
                k_r = work.tile([128, 512], F32R, tag="kr", name=f"kr{t}")
                nc.vector.tensor_scalar_max(k_r, k_ps, 0.0)
                v_sb = work.tile([128, MH], F32R, tag="vs", name=f"vs{t}")
                nc.vector.tensor_copy(v_sb, v_ps[:, :MH])

                # ksum: [2, 512] += sincos[128,2].T @ relu(k)[128,512]
                nc.tensor.matmul(
                    ksum_ps, sct_sb[:, 2 * t : 2 * t + 2], k_r,
                    start=t == 0, stop=t == NT - 1,
                )
                # kv: per d2-tile [128, 512] = [ (k*sin).T@v | (k*cos).T@v ]
                # The s-half and c-half share one PSUM bank (= one 2 KiB zero
                # region): the s-group's start clears the whole bank, so the
                # c-group never sets start, and only the c-group's final
                # matmul sets stop.
                for d2 in range(DT):
                    nc.tensor.matmul(
                        kv_ps[d2][:, 0:MH], k_s[:, ts(d2, 128)], v_sb,
                        start=t == 0, stop=False,
                    )
                    nc.tensor.matmul(
                        kv_ps[d2][:, MH:512], k_c[:, ts(d2, 128)], v_sb,
                        start=False, stop=t == NT - 1,
                    )

            # ---- leftover q-units (chunk 7) ----
            for u in range(NT - 4, NT):
                mt, nc2 = u % 4, u // 4
                q_ps = ppool.tile([128, 512], F32, tag="proj", name=f"q_ps{u}")
                for dt in range(DT):
                    nc.tensor.matmul(
                        q_ps, wq_sb[:, dt, ts(mt, 128)], xc[nc2][:, dt, :],
                        start=dt == 0, stop=dt == DT - 1,
                    )
                nc.scalar.activation(qc[nc2][:, mt, :], q_ps, AF.Relu)

            # ---- move kv/ksum to SBUF; transpose ksum to [d2-part, dt, 2] ----
            kv_sb = consts.tile([128, DT, 512], F32R)
            for d2 in range(DT):
                nc.vector.tensor_copy(kv_sb[:, d2, :], kv_ps[d2])
            ksum_row = work.tile([2, 512], F32R, tag="ksrow")
            nc.vector.tensor_copy(ksum_row, ksum_ps)
            ksum_sb = consts.tile([128, DT, 2], F32R)
            for d2 in range(DT):
                # reuse the kv bank slots (free after the kv_sb copies)
                tp = kvpool.tile([128, 2], F32R, tag=f"kv{d2}", name=f"tp{d2}")
                nc.tensor.transpose(tp, ksum_row[:, ts(d2, 128)], id_sb[0:2, 0:2])
                nc.vector.tensor_copy(ksum_sb[:, d2, :], tp)

            # ---- qs = q @ ksum, batched in transposed space ----
            # qsT[sc, n] accumulates with a cheap 2-column weight load, one
            # N=512 matmul per (chunk, d2) instead of 4 tiny matmuls per tile;
            # then 32 tiny PE transposes bring it back to [n-part, 2].
            qs_sb = consts.tile([128, NT, 2], F32)
            for c in range(8):
                qsT_ps = kvpool.tile([2, 512], F32, tag=f"kv{c % 2}", name=f"qsT{c}")
                for d2 in range(DT):
                    nc.tensor.matmul(
                        qsT_ps, ksum_sb[:, d2, :], qc[c][:, d2, :],
                        start=d2 == 0, stop=d2 == DT - 1,
                    )
                qs_row = work.tile([2, 512], F32R, tag="qsrow", name=f"qsrow{c}")
                nc.vector.tensor_copy(qs_row, qsT_ps)
                for j in range(4):
                    ti = 4 * c + j
                    tp2 = kvpool.tile(
                        [128, 2], F32R, tag=f"kv{2 + c % 2}", name=f"tp2_{ti}"
                    )
                    nc.tensor.transpose(tp2, qs_row[:, ts(j, 128)], id_sb[0:2, 0:2])
                    nc.vector.tensor_copy(qs_sb[:, ti, :], tp2)

            # ---- z for all tiles in a few wide DVE ops ----
            # zden[p, t] = sin*qs_s + cos*qs_c ; sz/cz = sin*z, cos*z
            sct3 = f32(sct_sb).rearrange("p (t two) -> p t two", two=2)
            qq_all = opool.tile([128, NT, 2], F32, tag="qqall")
            nc.vector.tensor_mul(qq_all, qs_sb, sct3)
            zden = opool.tile([128, NT], F32, tag="zden")
            nc.vector.tensor_reduce(zden, qq_all, axis=mybir.AxisListType.X, op=ALU.add)
            zclip = opool.tile([128, NT], F32, tag="zclip")
            nc.vector.tensor_scalar_max(zclip, zden, 1e-6)
            zt_all = opool.tile([128, NT], F32, tag="ztall")
            nc.vector.reciprocal(zt_all, zclip)
            sz_all = opool.tile([128, NT], F32, tag="szall")
            nc.vector.tensor_mul(sz_all, zt_all, sct3[:, :, 0])
            cz_all = opool.tile([128, NT], F32, tag="czall")
            nc.vector.tensor_mul(cz_all, zt_all, sct3[:, :, 1])

            # ---- output stage: out rows tile t ----
            for t in range(NT):
                qt = qc[t // 4]
                nslc = ts(t % 4, 128)

                o_ps = ppool.tile([128, 512], F32, tag="proj", name=f"o_ps{t}")
                for d2 in range(DT):
                    nc.tensor.matmul(
                        o_ps, qt[:, d2, nslc], kv_sb[:, d2, :],
                        start=d2 == 0, stop=d2 == DT - 1,
                    )

                # out = (sin*z)*o_s + (cos*z)*o_c
                resA = opool.tile([128, MH], F32, tag="resA", name=f"resA{t}")
                nc.scalar.activation(
                    resA, o_ps[:, 0:MH], AF.Copy, scale=sz_all[:, t : t + 1]
                )
                resB = opool.tile([128, MH], F32, tag="resB", name=f"resB{t}")
                nc.vector.tensor_scalar_mul(resB, o_ps[:, MH:512], cz_all[:, t : t + 1])
                res = opool.tile([128, MH], F32, tag="res", name=f"res{t}")
                nc.vector.tensor_add(res, resB, resA)

                h, s0 = t // 4, (t % 4) * 128
                nc.sync.dma_start(
                    out_r[ds(s0, 128), :, h, :],
                    res.rearrange("p (j hd) -> p j hd", hd=64),
                )

    nc.compile()
    return nc


_prog_cache = {}


def get_program():
    if "nc" not in _prog_cache:
        _prog_cache["nc"] = build_program()
    return _prog_cache["nc"]


def make_in_maps(x, Wq, Wk, Wv):
    x = np.ascontiguousarray(np.asarray(x, dtype=np.float32))
    Wq = np.asarray(Wq, dtype=np.float32)
    Wk = np.asarray(Wk, dtype=np.float32)
    Wv = np.asarray(Wv, dtype=np.float32)

    idx = (np.pi / 2) * np.arange(1, N + 1, dtype=np.float64) / N
    sin = np.sin(idx).astype(np.float32)
    cos = np.cos(idx).astype(np.float32)
    sct = np.empty((128, 2 * NT), dtype=np.float32)
    for t in range(NT):
        sct[:, 2 * t] = sin[t * 128 : (t + 1) * 128]
        sct[:, 2 * t + 1] = cos[t * 128 : (t + 1) * 128]
    ident = np.eye(128, dtype=np.float32)

    xT = np.ascontiguousarray(x.transpose(0, 2, 1))  # [B, D, N]
    WqT = np.ascontiguousarray(Wq.T)
    WkT = np.ascontiguousarray(Wk.T)
    WvT = np.ascontiguousarray(Wv.T)

    in_maps = []
    for core in range(8):
        b, J = core >> 1, core & 1
        in_maps.append(
            {
                "xT": xT[b],
                "wqT": WqT,
                "wkT": WkT,
                "wvT": np.ascontiguousarray(WvT[:, J * MH : (J + 1) * MH]),
                "sct": sct,
                "ident": ident,
            }
        )
    return in_maps


def assemble(results):
    out = np.empty((B, N, D), dtype=np.float32)
    for b in range(B):
        b0 = results[2 * b]["out"].reshape(512, 4, 512)
        b1 = results[2 * b + 1]["out"].reshape(512, 4, 512)
        out[b] = np.stack([b0, b1], axis=1).reshape(N, D)
    return out


def run(x, Wq, Wk, Wv, **spmd_kwargs):
    nc = get_program()
    in_maps = make_in_maps(x, Wq, Wk, Wv)
    res = run_bass_kernel_spmd(nc, in_maps, list(range(8)), **spmd_kwargs)
    return assemble(res.results), res


def kernel(x, Wq, Wk, Wv):
    out, _ = run(x, Wq, Wk, Wv)
    return out


# revision 9
# speedup vs baseline: 1.0035x; 1.0035x over previous
"""Cosformer multi-head attention kernel for 8 Trainium2 NeuronCores.

Reference computation (per batch b):
    q = relu(x @ Wq.T); k = relu(x @ Wk.T); v = x @ Wv.T
    q_ = [q*sin, q*cos]; k_ = [k*sin, k*cos]      (sin/cos indexed by position n)
    kv = k_.T @ v;  z = 1/clip(q_ @ sum_n k_, 1e-6)
    out = (q_ @ kv) * z   followed by a head-scrambling reshape.

Sharding: 8 cores = 4 batches x 2 halves of the output columns (m). Each core
computes q and k in full for its batch (needed for the full-feature contraction)
and v / kv / out only for its 256-column half. No collectives needed.

Decomposition used on-chip (avoids materializing the 2D-wide q_/k_):
    kv_s = (k*sin).T @ v ; kv_c = (k*cos).T @ v            [512, 256] each
    ksum_s = sum_n (k*sin) ; ksum_c = sum_n (k*cos)        [512]
    o_s = q @ kv_s ; o_c = q @ kv_c ; qs_s = q @ ksum_s ; qs_c = q @ ksum_c
    out = (sin*o_s + cos*o_c) / clip(sin*qs_s + cos*qs_c, 1e-6)

All matmuls run as float32r (FP22 multiplies, fp32 accumulate) which is 4x
faster than true fp32 on the PE array at free-dim >= 256. The walrus verifier
requires every tensor consumed by an FP32r matmul to be declared float32r at
its producer, so all matmul operands are float32r end-to-end (bit-compatible
with fp32 on the host side).

The final head-scramble permutation is folded into the output DMA access
pattern; the host only does cheap reshapes to reassemble the full output.
"""

import math
import numpy as np

import concourse.bass as bass
import concourse.mybir as mybir
import concourse.tile as tile
from concourse import bacc
from concourse.bass import ts, ds
from concourse.bass_utils import run_bass_kernel_spmd

B, N, D = 4, 4096, 512
MH = 256          # per-core output column half width
NT = N // 128     # 32 n-tiles
DT = D // 128     # 4 d-tiles
F32 = mybir.dt.float32
F32R = mybir.dt.float32r
AF = mybir.ActivationFunctionType
ALU = mybir.AluOpType


def f32(ap):
    """Read an f32r tensor as plain fp32 (same bits) for non-matmul ops."""
    return ap.bitcast(F32)


def build_program():
    nc = bacc.Bacc("TRN2", target_bir_lowering=False, debug=False, num_devices=8)

    xT = nc.dram_tensor("xT", [D, N], F32R, kind="ExternalInput").ap()
    wqT = nc.dram_tensor("wqT", [D, D], F32R, kind="ExternalInput").ap()
    wkT = nc.dram_tensor("wkT", [D, D], F32R, kind="ExternalInput").ap()
    wvT = nc.dram_tensor("wvT", [D, MH], F32R, kind="ExternalInput").ap()
    sct = nc.dram_tensor("sct", [128, 2 * NT], F32R, kind="ExternalInput").ap()
    ident = nc.dram_tensor("ident", [128, 128], F32R, kind="ExternalInput").ap()
    out = nc.dram_tensor("out", [N // 2, D], F32, kind="ExternalOutput").ap()

    # scrambled output view: tile rows rr = h*512 + s, cols cc = jj*64 + hd
    # land at out[s*4 + jj, h*64 + hd]
    out_r = out.rearrange("(s j) (h hd) -> s j h hd", j=4, hd=64)

    with tile.TileContext(nc) as tc:
        with (
            tc.tile_pool(name="consts", bufs=1) as consts,
            tc.tile_pool(name="xpool", bufs=1) as xpool,
            tc.tile_pool(name="qpool", bufs=1) as qpool,
            tc.tile_pool(name="work", bufs=2) as work,
            tc.tile_pool(name="opool", bufs=3) as opool,
            tc.tile_pool(name="ppool", bufs=3, space="PSUM") as ppool,
            tc.tile_pool(name="kvpool", bufs=1, space="PSUM") as kvpool,
        ):
            # ---- constants ----
            # Small/weight loads go on the ACT HWDGE ring, x chunks on the SP
            # ring, so the first k/v matmuls (need sct+wk+wv+xc0) start ASAP.
            sct_sb = consts.tile([128, 2 * NT], F32R)
            nc.scalar.dma_start(sct_sb, sct)
            wk_sb = consts.tile([128, DT, D], F32R)
            nc.scalar.dma_start(wk_sb, wkT.rearrange("(dt p) m -> p dt m", p=128))
            wv_sb = consts.tile([128, DT, MH], F32R)
            nc.scalar.dma_start(wv_sb, wvT.rearrange("(dt p) m -> p dt m", p=128))

            # ---- x, 8 chunks of [128, 4, 512] (1 MiB DMAs) ----
            xc = []
            xT_r = xT.rearrange("(dt p) n -> p dt n", p=128)
            for c in range(8):
                t_ = xpool.tile([128, DT, 512], F32R, tag=f"xc{c}", name=f"xc{c}")
                nc.sync.dma_start(t_, xT_r[:, :, ts(c, 512)])
                xc.append(t_)

            wq_sb = consts.tile([128, DT, D], F32R)
            nc.scalar.dma_start(wq_sb, wqT.rearrange("(dt p) m -> p dt m", p=128))
            id_sb = consts.tile([128, 128], F32R)
            nc.scalar.dma_start(id_sb, ident)

            # qT storage [m-part, mt, n] chunks
            qc = [
                qpool.tile([128, DT, 512], F32R, tag=f"qc{c}", name=f"qc{c}")
                for c in range(8)
            ]

            # persistent psum accumulators
            kv_ps = [
                kvpool.tile([128, 512], F32, tag=f"kv{d2}", name=f"kv{d2}")
                for d2 in range(DT)
            ]
            ksum_ps = kvpool.tile([2, 512], F32, tag="ksum", name="ksum")

            # ---- fused loop: k/v/q projections + kv/ksum accumulation ----
            for t in range(NT):
                xt = xc[t // 4]
                nslc = ts(t % 4, 128)  # n-tile slice inside the x chunk
                sin_ap = f32(sct_sb[:, 2 * t : 2 * t + 1])
                cos_ap = f32(sct_sb[:, 2 * t + 1 : 2 * t + 2])

                k_ps = ppool.tile([128, 512], F32, tag="proj", name=f"k_ps{t}")
                for dt in range(DT):
                    nc.tensor.matmul(
                        k_ps, xt[:, dt, nslc], wk_sb[:, dt, :],
                        start=dt == 0, stop=dt == DT - 1,
                    )
                v_ps = ppool.tile([128, 512], F32, tag="proj", name=f"v_ps{t}")
                for dt in range(DT):
                    nc.tensor.matmul(
                        v_ps[:, :MH], xt[:, dt, nslc], wv_sb[:, dt, :],
                        start=dt == 0, stop=dt == DT - 1,
                    )
                # q-units lag 4 tiles so the first tiles only need xc0+wk+wv
                # (earlier PE start while the later x chunks stream in).
                if t >= 4:
                    u = t - 4
                    mt, nc2 = u % 4, u // 4
                    q_ps = ppool.tile([128, 512], F32, tag="proj", name=f"q_ps{u}")
                    for dt in range(DT):
                        nc.tensor.matmul(
                            q_ps, wq_sb[:, dt, ts(mt, 128)], xc[nc2][:, dt, :],
                            start=dt == 0, stop=dt == DT - 1,
                        )
                    nc.scalar.activation(qc[nc2][:, mt, :], q_ps, AF.Relu)

                k_s = work.tile([128, 512], F32R, tag="ks", name=f"ks{t}")
                nc.scalar.activation(k_s, k_ps, AF.Relu, scale=sin_ap)
                k_c = work.tile([128, 512], F32R, tag="kc", name=f"kc{t}")
                nc.scalar.activation(k_c, k_ps, AF.Relu, scale=cos_ap)
                k_r = work.tile([128, 512], F32R, tag="kr", name=f"kr{t}")
                nc.vector.tensor_scalar_max(k_r, k_ps, 0.0)
                v_sb = work.tile([128, MH], F32R, tag="vs", name=f"vs{t}")
                nc.vector.tensor_copy(v_sb, v_ps[:, :MH])

                # ksum: [2, 512] += sincos[128,2].T @ relu(k)[128,512]
                nc.tensor.matmul(
                    ksum_ps, sct_sb[:, 2 * t : 2 * t + 2], k_r,
                    start=t == 0, stop=t == NT - 1,
                )
                # kv: per d2-tile [128, 512] = [ (k*sin).T@v | (k*cos).T@v ]
                # The s-half and c-half share one PSUM bank (= one 2 KiB zero
                # region): the s-group's start clears the whole bank, so the
                # c-group never sets start, and only the c-group's final
                # matmul sets stop.
                for d2 in range(DT):
                    nc.tensor.matmul(
                        kv_ps[d2][:, 0:MH], k_s[:, ts(d2, 128)], v_sb,
                        start=t == 0, stop=False,
                    )
                    nc.tensor.matmul(
                        kv_ps[d2][:, MH:512], k_c[:, ts(d2, 128)], v_sb,
                        start=False, stop=t == NT - 1,
                    )

            # ---- leftover q-units (chunk 7) ----
            for u in range(NT - 4, NT):
                mt, nc2 = u % 4, u // 4
                q_ps = ppool.tile([128, 512], F32, tag="proj", name=f"q_ps{u}")
                for dt in range(DT):
                    nc.tensor.matmul(
                        q_ps, wq_sb[:, dt, ts(mt, 128)], xc[nc2][:, dt, :],
                        start=dt == 0, stop=dt == DT - 1,
                    )
                nc.scalar.activation(qc[nc2][:, mt, :], q_ps, AF.Relu)

            # ---- move kv/ksum to SBUF; transpose ksum to [d2-part, dt, 2] ----
            kv_sb = consts.tile([128, DT, 512], F32R)
            for d2 in range(DT):
                nc.vector.tensor_copy(kv_sb[:, d2, :], kv_ps[d2])
            ksum_row = work.tile([2, 512], F32R, tag="ksrow")
            nc.vector.tensor_copy(ksum_row, ksum_ps)
            ksum_sb = consts.tile([128, DT, 2], F32R)
            for d2 in range(DT):
                # reuse the kv bank slots (free after the kv_sb copies)
                tp = kvpool.tile([128, 2], F32R, tag=f"kv{d2}", name=f"tp{d2}")
                nc.tensor.transpose(tp, ksum_row[:, ts(d2, 128)], id_sb[0:2, 0:2])
                nc.vector.tensor_copy(ksum_sb[:, d2, :], tp)

            # ---- output stage, per x-chunk (4 row tiles) ----
            # Per chunk: qs matmuls in transposed space (cheap 2-column weight
            # load), then the 16 bulk o-matmuls (hide the DVE ping-pong), then
            # tiny PE transposes + per-chunk z, then the 4 epilogues. o-PSUM
            # rotates through 6 slots (3 from ppool + the freed kv banks).
            sct3 = f32(sct_sb).rearrange("p (t two) -> p t two", two=2)
            for c in range(8):
                qsT_ps = kvpool.tile([2, 512], F32, tag="ksum", name=f"qsT{c}")
                for d2 in range(DT):
                    nc.tensor.matmul(
                        qsT_ps, ksum_sb[:, d2, :], qc[c][:, d2, :],
                        start=d2 == 0, stop=d2 == DT - 1,
                    )
                qs_row = work.tile([2, 512], F32R, tag="qsrow", name=f"qsrow{c}")
                nc.vector.tensor_copy(qs_row, qsT_ps)

                o_tiles = []
                for j in range(4):
                    t = 4 * c + j
                    tag = ("proj", "kv0", "proj", "kv1", "proj", "kv2")[t % 6]
                    pool = ppool if tag == "proj" else kvpool
                    o_ps = pool.tile([128, 512], F32, tag=tag, name=f"o_ps{t}")
                    for d2 in range(DT):
                        nc.tensor.matmul(
                            o_ps, qc[c][:, d2, ts(j, 128)], kv_sb[:, d2, :],
                            start=d2 == 0, stop=d2 == DT - 1,
                        )
                    o_tiles.append(o_ps)

                qs_c = opool.tile([128, 4, 2], F32, tag="qsc", name=f"qs_c{c}")
                for j in range(4):
                    tp2 = kvpool.tile(
                        [128, 2], F32R, tag="kv3", name=f"tp2_{4 * c + j}"
                    )
                    nc.tensor.transpose(tp2, qs_row[:, ts(j, 128)], id_sb[0:2, 0:2])
                    nc.vector.tensor_copy(qs_c[:, j, :], tp2)

                # z for the 4 tiles: z = 1/max(sin*qs_s + cos*qs_c, 1e-6)
                s3 = sct3[:, 4 * c : 4 * c + 4, :]
                qq = opool.tile([128, 4, 2], F32, tag="qq", name=f"qq{c}")
                nc.vector.tensor_mul(qq, qs_c, s3)
                zden = opool.tile([128, 4], F32, tag="zden", name=f"zden{c}")
                nc.vector.tensor_reduce(
                    zden, qq, axis=mybir.AxisListType.X, op=ALU.add
                )
                zclip = opool.tile([128, 4], F32, tag="zclip", name=f"zclip{c}")
                nc.vector.tensor_scalar_max(zclip, zden, 1e-6)
                zt = opool.tile([128, 4], F32, tag="ztl", name=f"ztl{c}")
                nc.vector.reciprocal(zt, zclip)
                sz = opool.tile([128, 4], F32, tag="szl", name=f"szl{c}")
                nc.vector.tensor_mul(sz, zt, s3[:, :, 0])
                cz = opool.tile([128, 4], F32, tag="czl", name=f"czl{c}")
                nc.vector.tensor_mul(cz, zt, s3[:, :, 1])

                # epilogues: out = (sin*z)*o_s + (cos*z)*o_c, scrambled store
                for j in range(4):
                    t = 4 * c + j
                    o_ps = o_tiles[j]
                    resA = opool.tile([128, MH], F32, tag="resA", name=f"resA{t}")
                    nc.scalar.activation(
                        resA, o_ps[:, 0:MH], AF.Copy, scale=sz[:, j : j + 1]
                    )
                    resB = opool.tile([128, MH], F32, tag="resB", name=f"resB{t}")
                    nc.vector.tensor_scalar_mul(
                        resB, o_ps[:, MH:512], cz[:, j : j + 1]
                    )
                    res = opool.tile([128, MH], F32, tag="res", name=f"res{t}")
                    nc.vector.tensor_add(res, resB, resA)

                    h, s0 = t // 4, (t % 4) * 128
                    nc.sync.dma_start(
                        out_r[ds(s0, 128), :, h, :],
                        res.rearrange("p (j hd) -> p j hd", hd=64),
                    )

    nc.compile()
    return nc


_prog_cache = {}


def get_program():
    if "nc" not in _prog_cache:
        _prog_cache["nc"] = build_program()
    return _prog_cache["nc"]


def make_in_maps(x, Wq, Wk, Wv):
    x = np.ascontiguousarray(np.asarray(x, dtype=np.float32))
    Wq = np.asarray(Wq, dtype=np.float32)
    Wk = np.asarray(Wk, dtype=np.float32)
    Wv = np.asarray(Wv, dtype=np.float32)

    idx = (np.pi / 2) * np.arange(1, N + 1, dtype=np.float64) / N
    sin = np.sin(idx).astype(np.float32)
    cos = np.cos(idx).astype(np.float32)
    sct = np.empty((128, 2 * NT), dtype=np.float32)
    for t in range(NT):
        sct[:, 2 * t] = sin[t * 128 : (t + 1) * 128]
        sct[:, 2 * t + 1] = cos[t * 128 : (t + 1) * 128]
    ident = np.eye(128, dtype=np.float32)

    xT = np.ascontiguousarray(x.transpose(0, 2, 1))  # [B, D, N]
    WqT = np.ascontiguousarray(Wq.T)
    WkT = np.ascontiguousarray(Wk.T)
    WvT = np.ascontiguousarray(Wv.T)

    in_maps = []
    for core in range(8):
        b, J = core >> 1, core & 1
        in_maps.append(
            {
                "xT": xT[b],
                "wqT": WqT,
                "wkT": WkT,
                "wvT": np.ascontiguousarray(WvT[:, J * MH : (J + 1) * MH]),
                "sct": sct,
                "ident": ident,
            }
        )
    return in_maps


def assemble(results):
    out = np.empty((B, N, D), dtype=np.float32)
    for b in range(B):
        b0 = results[2 * b]["out"].reshape(512, 4, 512)
        b1 = results[2 * b + 1]["out"].reshape(512, 4, 512)
        out[b] = np.stack([b0, b1], axis=1).reshape(N, D)
    return out


def run(x, Wq, Wk, Wv, **spmd_kwargs):
    nc = get_program()
    in_maps = make_in_maps(x, Wq, Wk, Wv)
    res = run_bass_kernel_spmd(nc, in_maps, list(range(8)), **spmd_kwargs)
    return assemble(res.results), res


def kernel(x, Wq, Wk, Wv):
    out, _ = run(x, Wq, Wk, Wv)
    return out


# revision 11
# speedup vs baseline: 1.0560x; 1.0524x over previous
"""Cosformer multi-head attention kernel for 8 Trainium2 NeuronCores.

Reference computation (per batch b):
    q = relu(x @ Wq.T); k = relu(x @ Wk.T); v = x @ Wv.T
    q_ = [q*sin, q*cos]; k_ = [k*sin, k*cos]      (sin/cos indexed by position n)
    kv = k_.T @ v;  z = 1/clip(q_ @ sum_n k_, 1e-6)
    out = (q_ @ kv) * z   followed by a head-scrambling reshape.

Sharding: 8 cores = 4 batches x 2 halves of the output columns (m). Each core
computes q and k in full for its batch (needed for the full-feature contraction)
and v / kv / out only for its 256-column half. No collectives needed.

Decomposition used on-chip (avoids materializing the 2D-wide q_/k_):
    kv_s = (k*sin).T @ v ; kv_c = (k*cos).T @ v            [512, 256] each
    ksum_s = sum_n (k*sin) ; ksum_c = sum_n (k*cos)        [512]
    o_s = q @ kv_s ; o_c = q @ kv_c ; qs_s = q @ ksum_s ; qs_c = q @ ksum_c
    out = (sin*o_s + cos*o_c) / clip(sin*qs_s + cos*qs_c, 1e-6)

All matmuls run as float32r (FP22 multiplies, fp32 accumulate) which is 4x
faster than true fp32 on the PE array at free-dim >= 256. The walrus verifier
requires every tensor consumed by an FP32r matmul to be declared float32r at
its producer, so all matmul operands are float32r end-to-end (bit-compatible
with fp32 on the host side).

The final head-scramble permutation is folded into the output DMA access
pattern; the host only does cheap reshapes to reassemble the full output.
"""

import math
import numpy as np
import ml_dtypes

import concourse.bass as bass
import concourse.mybir as mybir
import concourse.tile as tile
from concourse import bacc
from concourse.bass import ts, ds
from concourse.bass_utils import run_bass_kernel_spmd

B, N, D = 4, 4096, 512
MH = 256          # per-core output column half width
NT = N // 128     # 32 n-tiles
DT = D // 128     # 4 d-tiles
F32 = mybir.dt.float32
F32R = mybir.dt.float32r
BF16 = mybir.dt.bfloat16
import os
MM_DT = BF16 if os.environ.get("KERNEL_BF16") else F32R
AF = mybir.ActivationFunctionType
ALU = mybir.AluOpType


def f32(ap):
    """Read an f32r tensor as plain fp32 (same bits) for non-matmul ops."""
    return ap.bitcast(F32)


def build_program():
    nc = bacc.Bacc("TRN2", target_bir_lowering=False, debug=False, num_devices=8)

    xT = nc.dram_tensor("xT", [D, N], MM_DT, kind="ExternalInput").ap()
    wqT = nc.dram_tensor("wqT", [D, D], MM_DT, kind="ExternalInput").ap()
    wkT = nc.dram_tensor("wkT", [D, D], MM_DT, kind="ExternalInput").ap()
    wvT = nc.dram_tensor("wvT", [D, MH], MM_DT, kind="ExternalInput").ap()
    sct = nc.dram_tensor("sct", [128, 2 * NT], MM_DT, kind="ExternalInput").ap()
    sctf = nc.dram_tensor("sctf", [128, 2 * NT], F32, kind="ExternalInput").ap()
    ident = nc.dram_tensor("ident", [128, 128], MM_DT, kind="ExternalInput").ap()
    out = nc.dram_tensor("out", [N // 2, D], F32, kind="ExternalOutput").ap()

    # scrambled output view: tile rows rr = h*512 + s, cols cc = jj*64 + hd
    # land at out[s*4 + jj, h*64 + hd]
    out_r = out.rearrange("(s j) (h hd) -> s j h hd", j=4, hd=64)

    with tile.TileContext(nc) as tc:
        with (
            tc.tile_pool(name="consts", bufs=1) as consts,
            tc.tile_pool(name="xpool", bufs=1) as xpool,
            tc.tile_pool(name="qpool", bufs=1) as qpool,
            tc.tile_pool(name="work", bufs=2) as work,
            tc.tile_pool(name="opool", bufs=3) as opool,
            tc.tile_pool(name="ppool", bufs=3, space="PSUM") as ppool,
            tc.tile_pool(name="kvpool", bufs=1, space="PSUM") as kvpool,
        ):
            # ---- constants ----
            # Small/weight loads go on the ACT HWDGE ring, x chunks on the SP
            # ring, so the first k/v matmuls (need sct+wk+wv+xc0) start ASAP.
            sct_sb = consts.tile([128, 2 * NT], MM_DT)
            nc.scalar.dma_start(sct_sb, sct)
            sctf_sb = consts.tile([128, 2 * NT], F32)
            nc.scalar.dma_start(sctf_sb, sctf)
            wk_sb = consts.tile([128, DT, D], MM_DT)
            nc.scalar.dma_start(wk_sb, wkT.rearrange("(dt p) m -> p dt m", p=128))
            wv_sb = consts.tile([128, DT, MH], MM_DT)
            nc.scalar.dma_start(wv_sb, wvT.rearrange("(dt p) m -> p dt m", p=128))

            # ---- x, 8 chunks of [128, 4, 512] (1 MiB DMAs) ----
            xc = []
            xT_r = xT.rearrange("(dt p) n -> p dt n", p=128)
            for c in range(8):
                t_ = xpool.tile([128, DT, 512], MM_DT, tag=f"xc{c}", name=f"xc{c}")
                nc.sync.dma_start(t_, xT_r[:, :, ts(c, 512)])
                xc.append(t_)

            wq_sb = consts.tile([128, DT, D], MM_DT)
            nc.scalar.dma_start(wq_sb, wqT.rearrange("(dt p) m -> p dt m", p=128))
            id_sb = consts.tile([128, 128], MM_DT)
            nc.scalar.dma_start(id_sb, ident)

            # qT storage [m-part, mt, n] chunks
            qc = [
                qpool.tile([128, DT, 512], MM_DT, tag=f"qc{c}", name=f"qc{c}")
                for c in range(8)
            ]

            # persistent psum accumulators
            kv_ps = [
                kvpool.tile([128, 512], F32, tag=f"kv{d2}", name=f"kv{d2}")
                for d2 in range(DT)
            ]
            ksum_ps = kvpool.tile([2, 512], F32, tag="ksum", name="ksum")

            # ---- fused loop: k/v/q projections + kv/ksum accumulation ----
            for t in range(NT):
                xt = xc[t // 4]
                nslc = ts(t % 4, 128)  # n-tile slice inside the x chunk
                sin_ap = sctf_sb[:, 2 * t : 2 * t + 1]
                cos_ap = sctf_sb[:, 2 * t + 1 : 2 * t + 2]

                k_ps = ppool.tile([128, 512], F32, tag="proj", name=f"k_ps{t}")
                for dt in range(DT):
                    nc.tensor.matmul(
                        k_ps, xt[:, dt, nslc], wk_sb[:, dt, :],
                        start=dt == 0, stop=dt == DT - 1,
                    )
                v_ps = ppool.tile([128, 512], F32, tag="proj", name=f"v_ps{t}")
                for dt in range(DT):
                    nc.tensor.matmul(
                        v_ps[:, :MH], xt[:, dt, nslc], wv_sb[:, dt, :],
                        start=dt == 0, stop=dt == DT - 1,
                    )
                # q-units lag 4 tiles so the first tiles only need xc0+wk+wv
                # (earlier PE start while the later x chunks stream in).
                if t >= 4:
                    u = t - 4
                    mt, nc2 = u % 4, u // 4
                    q_ps = ppool.tile([128, 512], F32, tag="proj", name=f"q_ps{u}")
                    for dt in range(DT):
                        nc.tensor.matmul(
                            q_ps, wq_sb[:, dt, ts(mt, 128)], xc[nc2][:, dt, :],
                            start=dt == 0, stop=dt == DT - 1,
                        )
                    nc.scalar.activation(qc[nc2][:, mt, :], q_ps, AF.Relu)

                k_s = work.tile([128, 512], MM_DT, tag="ks", name=f"ks{t}")
                nc.scalar.activation(k_s, k_ps, AF.Relu, scale=sin_ap)
                k_c = work.tile([128, 512], MM_DT, tag="kc", name=f"kc{t}")
                nc.scalar.activation(k_c, k_ps, AF.Relu, scale=cos_ap)
                k_r = work.tile([128, 512], MM_DT, tag="kr", name=f"kr{t}")
                nc.vector.tensor_scalar_max(k_r, k_ps, 0.0)
                v_sb = work.tile([128, MH], MM_DT, tag="vs", name=f"vs{t}")
                nc.vector.tensor_copy(v_sb, v_ps[:, :MH])

                # ksum: [2, 512] += sincos[128,2].T @ relu(k)[128,512]
                nc.tensor.matmul(
                    ksum_ps, sct_sb[:, 2 * t : 2 * t + 2], k_r,
                    start=t == 0, stop=t == NT - 1,
                )
                # kv: per d2-tile [128, 512] = [ (k*sin).T@v | (k*cos).T@v ]
                # The s-half and c-half share one PSUM bank (= one 2 KiB zero
                # region): the s-group's start clears the whole bank, so the
                # c-group never sets start, and only the c-group's final
                # matmul sets stop.
                for d2 in range(DT):
                    nc.tensor.matmul(
                        kv_ps[d2][:, 0:MH], k_s[:, ts(d2, 128)], v_sb,
                        start=t == 0, stop=False,
                    )
                    nc.tensor.matmul(
                        kv_ps[d2][:, MH:512], k_c[:, ts(d2, 128)], v_sb,
                        start=False, stop=t == NT - 1,
                    )

            # ---- leftover q-units (chunk 7) ----
            for u in range(NT - 4, NT):
                mt, nc2 = u % 4, u // 4
                q_ps = ppool.tile([128, 512], F32, tag="proj", name=f"q_ps{u}")
                for dt in range(DT):
                    nc.tensor.matmul(
                        q_ps, wq_sb[:, dt, ts(mt, 128)], xc[nc2][:, dt, :],
                        start=dt == 0, stop=dt == DT - 1,
                    )
                nc.scalar.activation(qc[nc2][:, mt, :], q_ps, AF.Relu)

            # ---- move kv/ksum to SBUF; transpose ksum to [d2-part, dt, 2] ----
            kv_sb = consts.tile([128, DT, 512], MM_DT)
            for d2 in range(DT):
                nc.vector.tensor_copy(kv_sb[:, d2, :], kv_ps[d2])
            ksum_row = work.tile([2, 512], MM_DT, tag="ksrow")
            nc.vector.tensor_copy(ksum_row, ksum_ps)
            ksum_sb = consts.tile([128, DT, 2], MM_DT)
            for d2 in range(DT):
                # reuse the kv bank slots (free after the kv_sb copies)
                tp = kvpool.tile([128, 2], MM_DT, tag=f"kv{d2}", name=f"tp{d2}")
                nc.tensor.transpose(tp, ksum_row[:, ts(d2, 128)], id_sb[0:2, 0:2])
                nc.vector.tensor_copy(ksum_sb[:, d2, :], tp)

            # ---- output stage, per x-chunk (4 row tiles) ----
            # Per chunk: qs matmuls in transposed space (cheap 2-column weight
            # load), then the 16 bulk o-matmuls (hide the DVE ping-pong), then
            # tiny PE transposes + per-chunk z, then the 4 epilogues. o-PSUM
            # rotates through 6 slots (3 from ppool + the freed kv banks).
            sct3 = sctf_sb.rearrange("p (t two) -> p t two", two=2)
            for c in range(8):
                qsT_ps = kvpool.tile([2, 512], F32, tag="ksum", name=f"qsT{c}")
                for d2 in range(DT):
                    nc.tensor.matmul(
                        qsT_ps, ksum_sb[:, d2, :], qc[c][:, d2, :],
                        start=d2 == 0, stop=d2 == DT - 1,
                    )
                qs_row = work.tile([2, 512], MM_DT, tag="qsrow", name=f"qsrow{c}")
                nc.vector.tensor_copy(qs_row, qsT_ps)

                o_tiles = []
                for j in range(4):
                    t = 4 * c + j
                    tag = ("proj", "kv0", "proj", "kv1", "proj", "kv2")[t % 6]
                    pool = ppool if tag == "proj" else kvpool
                    o_ps = pool.tile([128, 512], F32, tag=tag, name=f"o_ps{t}")
                    for d2 in range(DT):
                        nc.tensor.matmul(
                            o_ps, qc[c][:, d2, ts(j, 128)], kv_sb[:, d2, :],
                            start=d2 == 0, stop=d2 == DT - 1,
                        )
                    o_tiles.append(o_ps)

                qs_c = opool.tile([128, 4, 2], F32, tag="qsc", name=f"qs_c{c}")
                for j in range(4):
                    tp2 = kvpool.tile(
                        [128, 2], MM_DT, tag="kv3", name=f"tp2_{4 * c + j}"
                    )
                    nc.tensor.transpose(tp2, qs_row[:, ts(j, 128)], id_sb[0:2, 0:2])
                    nc.vector.tensor_copy(qs_c[:, j, :], tp2)

                # z for the 4 tiles: z = 1/max(sin*qs_s + cos*qs_c, 1e-6)
                s3 = sct3[:, 4 * c : 4 * c + 4, :]
                qq = opool.tile([128, 4, 2], F32, tag="qq", name=f"qq{c}")
                nc.vector.tensor_mul(qq, qs_c, s3)
                zden = opool.tile([128, 4], F32, tag="zden", name=f"zden{c}")
                nc.vector.tensor_reduce(
                    zden, qq, axis=mybir.AxisListType.X, op=ALU.add
                )
                zclip = opool.tile([128, 4], F32, tag="zclip", name=f"zclip{c}")
                nc.vector.tensor_scalar_max(zclip, zden, 1e-6)
                zt = opool.tile([128, 4], F32, tag="ztl", name=f"ztl{c}")
                nc.vector.reciprocal(zt, zclip)
                sz = opool.tile([128, 4], F32, tag="szl", name=f"szl{c}")
                nc.vector.tensor_mul(sz, zt, s3[:, :, 0])
                cz = opool.tile([128, 4], F32, tag="czl", name=f"czl{c}")
                nc.vector.tensor_mul(cz, zt, s3[:, :, 1])

                # epilogues: out = (sin*z)*o_s + (cos*z)*o_c, scrambled store
                for j in range(4):
                    t = 4 * c + j
                    o_ps = o_tiles[j]
                    resA = opool.tile([128, MH], F32, tag="resA", name=f"resA{t}")
                    nc.scalar.activation(
                        resA, o_ps[:, 0:MH], AF.Copy, scale=sz[:, j : j + 1]
                    )
                    resB = opool.tile([128, MH], F32, tag="resB", name=f"resB{t}")
                    nc.vector.tensor_scalar_mul(
                        resB, o_ps[:, MH:512], cz[:, j : j + 1]
                    )
                    res = opool.tile([128, MH], F32, tag="res", name=f"res{t}")
                    nc.vector.tensor_add(res, resB, resA)

                    h, s0 = t // 4, (t % 4) * 128
                    nc.sync.dma_start(
                        out_r[ds(s0, 128), :, h, :],
                        res.rearrange("p (j hd) -> p j hd", hd=64),
                    )

    nc.compile()
    return nc


_prog_cache = {}


def get_program():
    if "nc" not in _prog_cache:
        _prog_cache["nc"] = build_program()
    return _prog_cache["nc"]


def make_in_maps(x, Wq, Wk, Wv):
    x = np.ascontiguousarray(np.asarray(x, dtype=np.float32))
    Wq = np.asarray(Wq, dtype=np.float32)
    Wk = np.asarray(Wk, dtype=np.float32)
    Wv = np.asarray(Wv, dtype=np.float32)

    idx = (np.pi / 2) * np.arange(1, N + 1, dtype=np.float64) / N
    sin = np.sin(idx).astype(np.float32)
    cos = np.cos(idx).astype(np.float32)
    sct = np.empty((128, 2 * NT), dtype=np.float32)
    for t in range(NT):
        sct[:, 2 * t] = sin[t * 128 : (t + 1) * 128]
        sct[:, 2 * t + 1] = cos[t * 128 : (t + 1) * 128]
    ident = np.eye(128, dtype=np.float32)

    xT = np.ascontiguousarray(x.transpose(0, 2, 1))  # [B, D, N]
    WqT = np.ascontiguousarray(Wq.T)
    WkT = np.ascontiguousarray(Wk.T)
    WvT = np.ascontiguousarray(Wv.T)

    np_mm = ml_dtypes.bfloat16 if MM_DT == BF16 else np.float32
    in_maps = []
    for core in range(8):
        b, J = core >> 1, core & 1
        in_maps.append(
            {
                "xT": np.ascontiguousarray(xT[b], dtype=np_mm),
                "wqT": WqT.astype(np_mm),
                "wkT": WkT.astype(np_mm),
                "wvT": np.ascontiguousarray(
                    WvT[:, J * MH : (J + 1) * MH], dtype=np_mm
                ),
                "sct": sct.astype(np_mm),
                "sctf": sct,
                "ident": ident.astype(np_mm),
            }
        )
    return in_maps


def assemble(results):
    out = np.empty((B, N, D), dtype=np.float32)
    for b in range(B):
        b0 = results[2 * b]["out"].reshape(512, 4, 512)
        b1 = results[2 * b + 1]["out"].reshape(512, 4, 512)
        out[b] = np.stack([b0, b1], axis=1).reshape(N, D)
    return out


def run(x, Wq, Wk, Wv, **spmd_kwargs):
    nc = get_program()
    in_maps = make_in_maps(x, Wq, Wk, Wv)
    res = run_bass_kernel_spmd(nc, in_maps, list(range(8)), **spmd_kwargs)
    return assemble(res.results), res


def kernel(x, Wq, Wk, Wv):
    out, _ = run(x, Wq, Wk, Wv)
    return out
